# revision 29
# baseline (speedup 1.0000x reference)
import os
import sys

for _p in ("/opt/trn_rl_repo", "/root/.axon_site/_ro/trn_rl_repo"):
    if os.path.isdir(_p) and _p not in sys.path:
        sys.path.insert(0, _p)

import ml_dtypes
import numpy as np

BATCH = 64
P = 784
D = 10000
L = 256
C = 10
NCORES = 8
NB = 2              # batch shards
ND = 4              # D shards
BC = BATCH // NB    # 32 images per core
DC = D // ND        # 2500 useful dims per core
DP = 2560           # padded dim width per core
GBLK = 7            # gather blocks of 128 rows (896 >= 784)
NIDX = P // 16      # 49 idx columns per image
KT = DP // 128      # 20 classify tiles
NCHUNK = 5          # 5 x 512 fp32 PSUM chunks

# CoreSim (offline validation only) enforces a semaphore-to-queue lock that
# rejects the multi-queue rotation real hardware runs fine with; validation
# scripts set this to 1 so every gather lands on queue 0.
N_GATHER_QUEUES = 4

_compiled = None


def _build_bass():
    import concourse.bacc as bacc
    import concourse.tile as tile
    from concourse import mybir

    fp32 = mybir.dt.float32
    bf16 = mybir.dt.bfloat16
    fp8 = mybir.dt.float8e4
    u16 = mybir.dt.uint16
    u32 = mybir.dt.uint32
    i16 = mybir.dt.int16

    nc = bacc.Bacc("TRN2", target_bir_lowering=False, debug=False,
                   enable_asserts=False, num_swdge_queues=4)

    lvl = nc.dram_tensor("lvl", [L, DP], fp8, kind="ExternalInput")
    dumw = nc.dram_tensor("dumw", [128, 1], i16, kind="ExternalInput")
    posx = nc.dram_tensor("posx", [GBLK * 128, DP], fp8,
                          kind="ExternalInput")
    iotw = nc.dram_tensor("iotw", [128, 8 * GBLK], i16, kind="ExternalInput")
    selw = nc.dram_tensor("selw", [128, BC * 2 * BC], fp8,
                          kind="ExternalInput")
    clsw = nc.dram_tensor("clsw", [128, KT * C], bf16, kind="ExternalInput")
    idxw = nc.dram_tensor("idxw", [128, BC * NIDX], i16,
                          kind="ExternalInput")
    identw = nc.dram_tensor("identw", [BC, BC], bf16, kind="ExternalInput")
    biasw = nc.dram_tensor("biasw", [BC, 1], fp32, kind="ExternalInput")
    out = nc.dram_tensor("logitT", [C, BC], fp32, kind="ExternalOutput")

    HDP = DP // 2
    QDP = DP // 4
    CHUNKS = [(i * 512, 512) for i in range(NCHUNK)]

    with tile.TileContext(nc) as tc:
        with (
            tc.tile_pool(name="const", bufs=1) as cpool,
            tc.tile_pool(name="gath", bufs=1) as gpool,
            tc.tile_pool(name="prod", bufs=1) as ppool,
            tc.tile_pool(name="misc", bufs=1) as mpool,
            tc.tile_pool(name="psum", bufs=1, space="PSUM") as psum,
            tc.tile_pool(name="psumt", bufs=2, space="PSUM") as psumt,
        ):
            # dummy gather first: absorbs the cold Q7 SWDGE program load
            # (~6us) while the idx/constant loads run.  Its index comes from
            # DRAM so no gpsimd memset (another cold Q7 launch) is needed.
            idx_dummy = cpool.tile([128, 1], i16)
            nc.sync.dma_start(idx_dummy[:], dumw.ap())
            g_dummy = cpool.tile([128, DP], fp8)
            nc.gpsimd.dma_gather(
                g_dummy[:].rearrange("p (n m) -> p n m", m=DP),
                lvl.ap(), idx_dummy[:],
                num_idxs=16, num_idxs_reg=16, elem_size=DP,
            )

            iot_sb = cpool.tile([128, 8 * GBLK], i16)
            nc.sync.dma_start(iot_sb[:], iotw.ap())

            IDXHEAD = 8 * NIDX
            idx_sb = cpool.tile([128, BC * NIDX], i16)
            nc.sync.dma_start(idx_sb[:, :IDXHEAD], idxw.ap()[:, :IDXHEAD])
            nc.sync.dma_start(idx_sb[:, IDXHEAD:], idxw.ap()[:, IDXHEAD:])

            # posx arrives through the SWDGE gather queues (4 block-aligned
            # iota-gathers, one per queue) instead of one big HWDGE load:
            # HWDGE bulk traffic at the head starves the gather rings,
            # while these ride the same FIFO the level gathers use.
            posx_sb = cpool.tile([128, GBLK * DP // 2], u16)
            posx3 = posx_sb[:].bitcast(fp8).rearrange("p (n m) -> p n m",
                                                      m=DP)
            for k, (bl0, bl1) in enumerate([(0, 2), (2, 4), (4, 6), (6, 7)]):
                nc.gpsimd.dma_gather(
                    posx3[:, bl0:bl1, :], posx.ap(),
                    iot_sb[:, 8 * bl0:8 * bl1],
                    num_idxs=128 * (bl1 - bl0),
                    num_idxs_reg=128 * (bl1 - bl0),
                    elem_size=DP, queue_num=k,
                )
            sel_sb = cpool.tile([128, BC * 2 * BC], fp8)
            nc.sync.dma_start(sel_sb[:], selw.ap())
            cls_sb = cpool.tile([128, KT * C], bf16)
            nc.sync.dma_start(cls_sb[:], clsw.ap())
            id_sb = cpool.tile([BC, BC], bf16)
            nc.sync.dma_start(id_sb[:], identw.ap())
            bias_t = cpool.tile([BC, 1], fp32)
            nc.sync.dma_start(bias_t[:], biasw.ap())

            # one PSUM tile per 512-wide chunk: dependency tracking is
            # tile-granular, so per-chunk tiles let the sign + classify of
            # chunk 0 start while later chunks still accumulate the last image
            bund_t = [psum.tile([BC, cn], fp32, name=f"bund{ci}")
                      for ci, (c0, cn) in enumerate(CHUNKS)]

            NGBUF = 6
            NPBUF = 4
            gbig = gpool.tile([128, NGBUF * GBLK * DP], fp8)
            prbig = ppool.tile([128, NPBUF * GBLK * DP], fp8)
            g_tiles = [gbig[:, i * GBLK * DP:(i + 1) * GBLK * DP]
                       for i in range(NGBUF)]
            pr_tiles = [prbig[:, i * GBLK * DP:(i + 1) * GBLK * DP]
                        for i in range(NPBUF)]

            # rows 784-895 (block 6, partitions 16-127) are never written by
            # the gathers; zero them once so the block-6 matmul adds nothing
            nc.vector.memset(
                gbig[:].bitcast(u16).rearrange("p (i w) -> p i w",
                                               w=GBLK * HDP)
                [:, :, 6 * HDP:7 * HDP], 0)

            # warm up TensorE (HAM) while the first gathers drain; the
            # garbage accumulation lands in bund_t[0] and is discarded by
            # image 0's start=True
            warm_ps = bund_t[0]
            warm_rhs = sel_sb[:].rearrange("p (n m) -> p n m", m=1024)
            wsel = sel_sb[:, 0:2 * BC].rearrange("p (t m) -> p t m", t=2)
            for w in range(20):
                nc.tensor.matmul(
                    warm_ps[:], wsel, warm_rhs[:, 0:2, 0:512],
                    start=(w == 0), stop=(w == 19),
                    perf_mode=mybir.MatmulPerfMode.DoubleRow,
                )

            # Head images (0-3) split their gather into three block-aligned
            # pieces spread across rings so all rings start immediately;
            # tail images (28-31) split on their OWN ring with piece-wise
            # XOR + matmul so the tail compute overlaps the final drains.
            # Middle images keep one gather per ring slot: fewer SWDGE
            # calls (emission fixed cost ~1us/call) pace the steady state.
            SPLITS = [(0, 2, 16), (2, 4, 16), (4, GBLK, 17)]
            HEAD = set(range(N_GATHER_QUEUES))
            TAIL = set(range(BC - N_GATHER_QUEUES, BC))

            for b in range(BC):
                g = g_tiles[b % NGBUF]
                pr = pr_tiles[b % NPBUF]
                g3 = g.rearrange("p (n m) -> p n m", m=DP)
                pr3 = pr.rearrange("p (n m) -> p n m", m=DP)
                gu = g.bitcast(u32).rearrange("p (n m) -> p n m", m=QDP)
                pru = pr.bitcast(u32).rearrange("p (n m) -> p n m", m=QDP)
                posu = posx_sb[:].bitcast(u32).rearrange(
                    "p (n m) -> p n m", m=QDP)

                sel_b = sel_sb[:, b * 2 * BC:(b + 1) * 2 * BC]
                sel3 = sel_b.rearrange("p (t m) -> p t m", t=2)

                def bund_mm(j, start, chunks=None):
                    # DoubleRow pass over block pair (2j, 2j+1), j in 0..2;
                    # j == 3 is the single-block pass over block 6
                    for ci, (c0, cn) in chunks or list(enumerate(CHUNKS)):
                        if j < 3:
                            nc.tensor.matmul(
                                bund_t[ci][:], sel3,
                                pr3[:, 2 * j:2 * j + 2, c0:c0 + cn],
                                start=start, stop=False,
                                perf_mode=mybir.MatmulPerfMode.DoubleRow,
                            )
                        else:
                            nc.tensor.matmul(
                                bund_t[ci][:], sel3[:, 0, :],
                                pr3[:, 6, c0:c0 + cn],
                                start=False, stop=(b == BC - 1),
                            )

                if b in HEAD:
                    # queue (3b+k)%4: all four rings receive work within the
                    # first four emissions, and each queue still totals
                    # exactly 7 head blocks
                    col = b * NIDX
                    for k, (bl0, bl1, ncols) in enumerate(SPLITS):
                        nc.gpsimd.dma_gather(
                            g3[:, bl0:bl1, :], lvl.ap(),
                            idx_sb[:, col:col + ncols],
                            num_idxs=16 * ncols, num_idxs_reg=16 * ncols,
                            elem_size=DP,
                            queue_num=(3 * b + k) % N_GATHER_QUEUES,
                        )
                        col += ncols
                        nc.vector.tensor_tensor(
                            pru[:, bl0:bl1], gu[:, bl0:bl1],
                            posu[:, bl0:bl1],
                            op=mybir.AluOpType.bitwise_xor,
                        )
                        if k == 0:
                            bund_mm(0, b == 0)
                        elif k == 1:
                            bund_mm(1, False)
                        else:
                            bund_mm(2, False)
                            bund_mm(3, False)
                elif b in TAIL:
                    col = b * NIDX
                    for k, (bl0, bl1, ncols) in enumerate(SPLITS):
                        nc.gpsimd.dma_gather(
                            g3[:, bl0:bl1, :], lvl.ap(),
                            idx_sb[:, col:col + ncols],
                            num_idxs=16 * ncols, num_idxs_reg=16 * ncols,
                            elem_size=DP,
                            queue_num=b % N_GATHER_QUEUES,
                        )
                        col += ncols
                        nc.vector.tensor_tensor(
                            pru[:, bl0:bl1], gu[:, bl0:bl1],
                            posu[:, bl0:bl1],
                            op=mybir.AluOpType.bitwise_xor,
                        )
                        if k == 0:
                            bund_mm(0, False)
                        elif k == 1:
                            bund_mm(1, False)
                        elif b != BC - 1:
                            bund_mm(2, False)
                            bund_mm(3, False)
                        else:
                            # last image: chunk-major so each chunk's PSUM
                            # closes (stop) as early as possible and its
                            # sign + classify overlap the remaining chunks
                            for ci, (c0, cn) in enumerate(CHUNKS):
                                bund_mm(2, False, chunks=[(ci, (c0, cn))])
                                bund_mm(3, False, chunks=[(ci, (c0, cn))])
                else:
                    nc.gpsimd.dma_gather(
                        g3[:, :GBLK, :], lvl.ap(),
                        idx_sb[:, b * NIDX:(b + 1) * NIDX],
                        num_idxs=P, num_idxs_reg=P, elem_size=DP,
                        queue_num=b % N_GATHER_QUEUES,
                    )
                    nc.vector.tensor_tensor(
                        pr.bitcast(u32), g.bitcast(u32),
                        posx_sb[:].bitcast(u32),
                        op=mybir.AluOpType.bitwise_xor,
                    )
                    for j in range(4):
                        bund_mm(j, b == 0 and j == 0)

            # per-chunk sign + classify so chunk 0's tail work overlaps
            # the final accumulation of the later chunks
            enc_t = [mpool.tile([BC, cn], bf16, name=f"enc{ci}")
                     for ci, (c0, cn) in enumerate(CHUNKS)]
            for ci in range(NCHUNK):
                nc.scalar.activation(enc_t[ci][:], bund_t[ci][:],
                                     mybir.ActivationFunctionType.Sign,
                                     bias=bias_t[:])

            logit_ps = psum.tile([C, BC], fp32)
            for kt in range(KT):
                ci, kl = divmod(kt, 4)
                tp = psumt.tile([128, BC], bf16, name="tp")
                nc.tensor.transpose(
                    tp[:], enc_t[ci][:, kl * 128:(kl + 1) * 128], id_sb[:])
                etc = mpool.tile([128, BC], bf16, name="etc", bufs=4)
                nc.scalar.copy(etc[:], tp[:])
                nc.tensor.matmul(
                    logit_ps[:], cls_sb[:, kt * C:(kt + 1) * C], etc[:],
                    start=(kt == 0), stop=(kt == KT - 1),
                )

            logit_sb = mpool.tile([C, BC], fp32)
            nc.scalar.copy(logit_sb[:], logit_ps[:])
            nc.sync.dma_start(out.ap(), logit_sb[:])

    nc.compile()
    return nc


def _prep_inputs(x, position, level_weight, classify_weight):
    xf = x.reshape(BATCH, P).astype(np.float32)
    idx = np.clip(np.round(xf * np.float32(L - 1)), 0, L - 1).astype(np.int16)

    sel = np.zeros((128, BC, 2, BC), np.float32)
    for b in range(BC):
        sel[:, b, :, b] = 1.0
    selw = sel.reshape(128, BC * 2 * BC).astype(ml_dtypes.float8_e4m3)

    identw = np.eye(BC, dtype=np.float32).astype(ml_dtypes.bfloat16)

    idxw_h = []
    for h in range(NB):
        idxh = idx[h * BC:(h + 1) * BC]
        w = np.ascontiguousarray(
            idxh.reshape(BC, NIDX, 16).transpose(2, 0, 1)
        ).reshape(16, BC * NIDX)
        idxw_h.append(np.tile(w, (8, 1)))

    lvl_q, posx_q, clsw_q = [], [], []
    for q in range(ND):
        cols = slice(q * DC, (q + 1) * DC)

        lvl = np.zeros((L, DP), ml_dtypes.float8_e4m3)
        lvl[:, :DC] = level_weight[:, cols].astype(ml_dtypes.float8_e4m3)
        lvl_q.append(lvl)

        pos = np.zeros((GBLK * 128, DP), np.float32)
        pos[:P, :DC] = position[:, cols]
        signs = (pos < 0).astype(np.uint8) << 7
        posx_q.append(signs.view(ml_dtypes.float8_e4m3))

        cls = np.zeros((C, DP), np.float32)
        cls[:, :DC] = classify_weight[:, cols]
        clsw_q.append(np.ascontiguousarray(
            cls.reshape(C, KT, 128).transpose(2, 1, 0)
        ).reshape(128, KT * C).astype(ml_dtypes.bfloat16))

    iotw = np.tile(
        (np.arange(8 * GBLK)[None, :] * 16
         + np.arange(16)[:, None]).astype(np.int16), (8, 1))

    in_maps = []
    for h in range(NB):
        for q in range(ND):
            in_maps.append({
                "lvl": lvl_q[q],
                "dumw": np.zeros((128, 1), np.int16),
                "iotw": iotw,
                "posx": posx_q[q],
                "selw": selw,
                "clsw": clsw_q[q],
                "idxw": idxw_h[h],
                "identw": identw,
                "biasw": np.full((BC, 1), -0.5, np.float32),
            })
    return in_maps


def kernel(x, position, level_weight, classify_weight, _run_kwargs=None):
    global _compiled
    if _compiled is None:
        _compiled = _build_bass()
    nc = _compiled

    import concourse.bass_utils as bass_utils

    in_maps = _prep_inputs(x, position, level_weight, classify_weight)
    res = bass_utils.run_bass_kernel_spmd(
        nc, in_maps, core_ids=list(range(NCORES)), **(_run_kwargs or {})
    )
    logit = np.zeros((BATCH, C), np.float32)
    for h in range(NB):
        for q in range(ND):
            logit[h * BC:(h + 1) * BC] += \
                res.results[h * ND + q]["logitT"].T.astype(np.float32)
    kernel.last_result = res
    return logit


# revision 31
# speedup vs baseline: 1.0383x; 1.0383x over previous
import os
import sys

for _p in ("/opt/trn_rl_repo", "/root/.axon_site/_ro/trn_rl_repo"):
    if os.path.isdir(_p) and _p not in sys.path:
        sys.path.insert(0, _p)

import ml_dtypes
import numpy as np

BATCH = 64
P = 784
D = 10000
L = 256
C = 10
NCORES = 8
NB = 2              # batch shards
ND = 4              # D shards
BC = BATCH // NB    # 32 images per core
DC = D // ND        # 2500 useful dims per core
DP = 2560           # padded dim width per core
GBLK = 7            # gather blocks of 128 rows (896 >= 784)
NIDX = P // 16      # 49 idx columns per image
KT = DP // 128      # 20 classify tiles
NCHUNK = 5          # 5 x 512 fp32 PSUM chunks

# CoreSim (offline validation only) enforces a semaphore-to-queue lock that
# rejects the multi-queue rotation real hardware runs fine with; validation
# scripts set this to 1 so every gather lands on queue 0.
N_GATHER_QUEUES = 4

_compiled = None


def _build_bass():
    import concourse.bacc as bacc
    import concourse.tile as tile
    from concourse import mybir

    fp32 = mybir.dt.float32
    bf16 = mybir.dt.bfloat16
    fp8 = mybir.dt.float8e4
    u16 = mybir.dt.uint16
    u32 = mybir.dt.uint32
    i16 = mybir.dt.int16

    nc = bacc.Bacc("TRN2", target_bir_lowering=False, debug=False,
                   enable_asserts=False, num_swdge_queues=4)

    lvl = nc.dram_tensor("lvl", [L, DP], fp8, kind="ExternalInput")
    dumw = nc.dram_tensor("dumw", [128, 1], i16, kind="ExternalInput")
    posx = nc.dram_tensor("posx", [GBLK * 128, DP], fp8,
                          kind="ExternalInput")
    iotw = nc.dram_tensor("iotw", [128, 8 * GBLK], i16, kind="ExternalInput")
    selw = nc.dram_tensor("selw", [128, BC * 2 * BC], fp8,
                          kind="ExternalInput")
    clsw = nc.dram_tensor("clsw", [128, KT * C], bf16, kind="ExternalInput")
    idxw = nc.dram_tensor("idxw", [128, BC * NIDX], i16,
                          kind="ExternalInput")
    identw = nc.dram_tensor("identw", [BC, BC], bf16, kind="ExternalInput")
    biasw = nc.dram_tensor("biasw", [BC, 1], fp32, kind="ExternalInput")
    out = nc.dram_tensor("logitT", [C, BC], fp32, kind="ExternalOutput")

    HDP = DP // 2
    QDP = DP // 4
    CHUNKS = [(i * 512, 512) for i in range(NCHUNK)]

    with tile.TileContext(nc) as tc:
        with (
            tc.tile_pool(name="const", bufs=1) as cpool,
            tc.tile_pool(name="gath", bufs=1) as gpool,
            tc.tile_pool(name="prod", bufs=1) as ppool,
            tc.tile_pool(name="misc", bufs=1) as mpool,
            tc.tile_pool(name="psum", bufs=1, space="PSUM") as psum,
            tc.tile_pool(name="psumt", bufs=2, space="PSUM") as psumt,
        ):
            # dummy gather first: absorbs the cold Q7 SWDGE program load
            # (~6us) while the idx/constant loads run.  Its index comes from
            # DRAM so no gpsimd memset (another cold Q7 launch) is needed.
            idx_dummy = cpool.tile([128, 1], i16)
            nc.sync.dma_start(idx_dummy[:], dumw.ap())
            g_dummy = cpool.tile([128, DP], fp8)
            nc.gpsimd.dma_gather(
                g_dummy[:].rearrange("p (n m) -> p n m", m=DP),
                lvl.ap(), idx_dummy[:],
                num_idxs=16, num_idxs_reg=16, elem_size=DP,
            )

            iot_sb = cpool.tile([128, 8 * GBLK], i16)
            nc.sync.dma_start(iot_sb[:], iotw.ap())

            IDXHEAD = 8 * NIDX
            idx_sb = cpool.tile([128, BC * NIDX], i16)
            nc.sync.dma_start(idx_sb[:, :IDXHEAD], idxw.ap()[:, :IDXHEAD])
            nc.sync.dma_start(idx_sb[:, IDXHEAD:], idxw.ap()[:, IDXHEAD:])

            # posx arrives through the SWDGE gather queues (4 block-aligned
            # iota-gathers, one per queue) instead of one big HWDGE load:
            # HWDGE bulk traffic at the head starves the gather rings,
            # while these ride the same FIFO the level gathers use.
            posx_sb = cpool.tile([128, GBLK * DP // 2], u16)
            posx3 = posx_sb[:].bitcast(fp8).rearrange("p (n m) -> p n m",
                                                      m=DP)
            for k, (bl0, bl1) in enumerate([(0, 2), (2, 4), (4, 6), (6, 7)]):
                nc.gpsimd.dma_gather(
                    posx3[:, bl0:bl1, :], posx.ap(),
                    iot_sb[:, 8 * bl0:8 * bl1],
                    num_idxs=128 * (bl1 - bl0),
                    num_idxs_reg=128 * (bl1 - bl0),
                    elem_size=DP, queue_num=k,
                )
            sel_sb = cpool.tile([128, BC * 2 * BC], fp8)
            nc.sync.dma_start(sel_sb[:], selw.ap())
            cls_sb = cpool.tile([128, KT * C], bf16)
            nc.sync.dma_start(cls_sb[:], clsw.ap())
            id_sb = cpool.tile([BC, BC], bf16)
            nc.sync.dma_start(id_sb[:], identw.ap())
            bias_t = cpool.tile([BC, 1], fp32)
            nc.sync.dma_start(bias_t[:], biasw.ap())

            # one PSUM tile per 512-wide chunk: dependency tracking is
            # tile-granular, so per-chunk tiles let the sign + classify of
            # chunk 0 start while later chunks still accumulate the last image
            bund_t = [psum.tile([BC, cn], fp32, name=f"bund{ci}")
                      for ci, (c0, cn) in enumerate(CHUNKS)]

            NGBUF = 6
            NPBUF = 4
            gbig = gpool.tile([128, NGBUF * GBLK * DP], fp8)
            prbig = ppool.tile([128, NPBUF * GBLK * DP], fp8)
            g_tiles = [gbig[:, i * GBLK * DP:(i + 1) * GBLK * DP]
                       for i in range(NGBUF)]
            pr_tiles = [prbig[:, i * GBLK * DP:(i + 1) * GBLK * DP]
                        for i in range(NPBUF)]

            # rows 784-895 (block 6, partitions 16-127) are never written by
            # the gathers; zero them once so the block-6 matmul adds nothing
            nc.vector.memset(
                gbig[:].bitcast(u16).rearrange("p (i w) -> p i w",
                                               w=GBLK * HDP)
                [:, :, 6 * HDP:7 * HDP], 0)

            # warm up TensorE (HAM) while the first gathers drain; the
            # garbage accumulation lands in bund_t[0] and is discarded by
            # image 0's start=True
            warm_ps = bund_t[0]
            warm_rhs = sel_sb[:].rearrange("p (n m) -> p n m", m=1024)
            wsel = sel_sb[:, 0:2 * BC].rearrange("p (t m) -> p t m", t=2)
            for w in range(20):
                nc.tensor.matmul(
                    warm_ps[:], wsel, warm_rhs[:, 0:2, 0:512],
                    start=(w == 0), stop=(w == 19),
                    perf_mode=mybir.MatmulPerfMode.DoubleRow,
                )

            # Head images (0-3) split their gather into three block-aligned
            # pieces spread across rings so all rings start immediately;
            # tail images (28-31) split on their OWN ring with piece-wise
            # XOR + matmul so the tail compute overlaps the final drains.
            # Middle images keep one gather per ring slot: fewer SWDGE
            # calls (emission fixed cost ~1us/call) pace the steady state.
            SPLITS = [(0, 2, 16), (2, 4, 16), (4, GBLK, 17)]
            HEAD = set(range(N_GATHER_QUEUES))
            TAIL = set(range(BC - N_GATHER_QUEUES, BC))

            for b in range(BC):
                g = g_tiles[b % NGBUF]
                pr = pr_tiles[b % NPBUF]
                g3 = g.rearrange("p (n m) -> p n m", m=DP)
                pr3 = pr.rearrange("p (n m) -> p n m", m=DP)
                gu = g.bitcast(u32).rearrange("p (n m) -> p n m", m=QDP)
                pru = pr.bitcast(u32).rearrange("p (n m) -> p n m", m=QDP)
                posu = posx_sb[:].bitcast(u32).rearrange(
                    "p (n m) -> p n m", m=QDP)

                sel_b = sel_sb[:, b * 2 * BC:(b + 1) * 2 * BC]
                sel3 = sel_b.rearrange("p (t m) -> p t m", t=2)

                def bund_mm(j, start, chunks=None):
                    # DoubleRow pass over block pair (2j, 2j+1), j in 0..2;
                    # j == 3 is the single-block pass over block 6
                    for ci, (c0, cn) in chunks or list(enumerate(CHUNKS)):
                        if j < 3:
                            nc.tensor.matmul(
                                bund_t[ci][:], sel3,
                                pr3[:, 2 * j:2 * j + 2, c0:c0 + cn],
                                start=start, stop=False,
                                perf_mode=mybir.MatmulPerfMode.DoubleRow,
                            )
                        else:
                            nc.tensor.matmul(
                                bund_t[ci][:], sel3[:, 0, :],
                                pr3[:, 6, c0:c0 + cn],
                                start=False, stop=(b == BC - 1),
                            )

                if b in HEAD:
                    # queue (3b+k)%4: all four rings receive work within the
                    # first four emissions, and each queue still totals
                    # exactly 7 head blocks
                    col = b * NIDX
                    for k, (bl0, bl1, ncols) in enumerate(SPLITS):
                        nc.gpsimd.dma_gather(
                            g3[:, bl0:bl1, :], lvl.ap(),
                            idx_sb[:, col:col + ncols],
                            num_idxs=16 * ncols, num_idxs_reg=16 * ncols,
                            elem_size=DP,
                            queue_num=(3 * b + k) % N_GATHER_QUEUES,
                        )
                        col += ncols
                        nc.vector.tensor_tensor(
                            pru[:, bl0:bl1], gu[:, bl0:bl1],
                            posu[:, bl0:bl1],
                            op=mybir.AluOpType.bitwise_xor,
                        )
                        if k == 0:
                            bund_mm(0, b == 0)
                        elif k == 1:
                            bund_mm(1, False)
                        else:
                            bund_mm(2, False)
                            bund_mm(3, False)
                elif b in TAIL:
                    col = b * NIDX
                    for k, (bl0, bl1, ncols) in enumerate(SPLITS):
                        nc.gpsimd.dma_gather(
                            g3[:, bl0:bl1, :], lvl.ap(),
                            idx_sb[:, col:col + ncols],
                            num_idxs=16 * ncols, num_idxs_reg=16 * ncols,
                            elem_size=DP,
                            queue_num=b % N_GATHER_QUEUES,
                        )
                        col += ncols
                        nc.vector.tensor_tensor(
                            pru[:, bl0:bl1], gu[:, bl0:bl1],
                            posu[:, bl0:bl1],
                            op=mybir.AluOpType.bitwise_xor,
                        )
                        if k == 0:
                            bund_mm(0, False)
                        elif k == 1:
                            bund_mm(1, False)
                        elif b != BC - 1:
                            bund_mm(2, False)
                            bund_mm(3, False)
                        else:
                            # last image: chunk-major so each chunk's PSUM
                            # closes (stop) as early as possible and its
                            # sign + classify overlap the remaining chunks
                            for ci, (c0, cn) in enumerate(CHUNKS):
                                bund_mm(2, False, chunks=[(ci, (c0, cn))])
                                bund_mm(3, False, chunks=[(ci, (c0, cn))])
                else:
                    nc.gpsimd.dma_gather(
                        g3[:, :GBLK, :], lvl.ap(),
                        idx_sb[:, b * NIDX:(b + 1) * NIDX],
                        num_idxs=P, num_idxs_reg=P, elem_size=DP,
                        queue_num=b % N_GATHER_QUEUES,
                    )
                    nc.vector.tensor_tensor(
                        pr.bitcast(u32), g.bitcast(u32),
                        posx_sb[:].bitcast(u32),
                        op=mybir.AluOpType.bitwise_xor,
                    )
                    for j in range(4):
                        bund_mm(j, b == 0 and j == 0)

            # per-chunk sign + classify so chunk 0's tail work overlaps
            # the final accumulation of the later chunks
            enc_t = [mpool.tile([BC, cn], bf16, name=f"enc{ci}")
                     for ci, (c0, cn) in enumerate(CHUNKS)]
            for ci in range(NCHUNK):
                nc.scalar.activation(enc_t[ci][:], bund_t[ci][:],
                                     mybir.ActivationFunctionType.Sign,
                                     bias=bias_t[:])

            logit_ps = psum.tile([C, BC], fp32)
            for kt in range(KT):
                ci, kl = divmod(kt, 4)
                tp = psumt.tile([128, BC], bf16, name="tp")
                nc.tensor.transpose(
                    tp[:], enc_t[ci][:, kl * 128:(kl + 1) * 128], id_sb[:])
                etc = mpool.tile([128, BC], bf16, name="etc", bufs=4)
                nc.scalar.copy(etc[:], tp[:])
                nc.tensor.matmul(
                    logit_ps[:], cls_sb[:, kt * C:(kt + 1) * C], etc[:],
                    start=(kt == 0), stop=(kt == KT - 1),
                )

            logit_sb = mpool.tile([C, BC], fp32)
            nc.scalar.copy(logit_sb[:], logit_ps[:])
            nc.sync.dma_start(out.ap(), logit_sb[:])

    nc.compile()
    return nc


def _prep_inputs(x, position, level_weight, classify_weight):
    xf = x.reshape(BATCH, P).astype(np.float32)
    idx = np.clip(np.round(xf * np.float32(L - 1)), 0, L - 1).astype(np.int16)

    sel = np.zeros((128, BC, 2, BC), np.float32)
    for b in range(BC):
        sel[:, b, :, b] = 1.0
    selw = sel.reshape(128, BC * 2 * BC).astype(ml_dtypes.float8_e4m3)

    identw = np.eye(BC, dtype=np.float32).astype(ml_dtypes.bfloat16)

    idxw_h = []
    for h in range(NB):
        idxh = idx[h * BC:(h + 1) * BC]
        w = np.ascontiguousarray(
            idxh.reshape(BC, NIDX, 16).transpose(2, 0, 1)
        ).reshape(16, BC * NIDX)
        idxw_h.append(np.tile(w, (8, 1)))

    lvl_q, posx_q, clsw_q = [], [], []
    for q in range(ND):
        cols = slice(q * DC, (q + 1) * DC)

        lvl = np.zeros((L, DP), ml_dtypes.float8_e4m3)
        lvl[:, :DC] = level_weight[:, cols].astype(ml_dtypes.float8_e4m3)
        lvl_q.append(lvl)

        pos = np.zeros((GBLK * 128, DP), np.float32)
        pos[:P, :DC] = position[:, cols]
        signs = (pos < 0).astype(np.uint8) << 7
        posx_q.append(signs.view(ml_dtypes.float8_e4m3))

        cls = np.zeros((C, DP), np.float32)
        cls[:, :DC] = classify_weight[:, cols]
        clsw_q.append(np.ascontiguousarray(
            cls.reshape(C, KT, 128).transpose(2, 1, 0)
        ).reshape(128, KT * C).astype(ml_dtypes.bfloat16))

    iotw = np.tile(
        (np.arange(8 * GBLK)[None, :] * 16
         + np.arange(16)[:, None]).astype(np.int16), (8, 1))

    in_maps = []
    for h in range(NB):
        for q in range(ND):
            in_maps.append({
                "lvl": lvl_q[q],
                "dumw": np.zeros((128, 1), np.int16),
                "iotw": iotw,
                "posx": posx_q[q],
                "selw": selw,
                "clsw": clsw_q[q],
                "idxw": idxw_h[h],
                "identw": identw,
                "biasw": np.full((BC, 1), -0.5, np.float32),
            })
    return in_maps


def kernel(x, position, level_weight, classify_weight, _run_kwargs=None):
    global _compiled
    if _compiled is None:
        _compiled = _build_bass()
    nc = _compiled

    import concourse.bass_utils as bass_utils

    in_maps = _prep_inputs(x, position, level_weight, classify_weight)
    res = bass_utils.run_bass_kernel_spmd(
        nc, in_maps, core_ids=list(range(NCORES)), **(_run_kwargs or {})
    )
    logit = np.zeros((BATCH, C), np.float32)
    for h in range(NB):
        for q in range(ND):
            logit[h * BC:(h + 1) * BC] += \
                res.results[h * ND + q]["logitT"].T.astype(np.float32)
    kernel.last_result = res
    return logit


# revision 33
# speedup vs baseline: 1.0806x; 1.0407x over previous
import os
import sys

for _p in ("/opt/trn_rl_repo", "/root/.axon_site/_ro/trn_rl_repo"):
    if os.path.isdir(_p) and _p not in sys.path:
        sys.path.insert(0, _p)

import ml_dtypes
import numpy as np

BATCH = 64
P = 784
D = 10000
L = 256
C = 10
NCORES = 8
NB = 2              # batch shards
ND = 4              # D shards
BC = BATCH // NB    # 32 images per core
DC = D // ND        # 2500 useful dims per core
DP = 2560           # padded dim width per core
GBLK = 7            # gather blocks of 128 rows (896 >= 784)
NIDX = P // 16      # 49 idx columns per image
KT = DP // 128      # 20 classify tiles
NCHUNK = 5          # 5 x 512 fp32 PSUM chunks

# CoreSim (offline validation only) enforces a semaphore-to-queue lock that
# rejects the multi-queue rotation real hardware runs fine with; validation
# scripts set this to 1 so every gather lands on queue 0.
N_GATHER_QUEUES = 4

_compiled = None


def _build_bass():
    import concourse.bacc as bacc
    import concourse.tile as tile
    from concourse import mybir

    fp32 = mybir.dt.float32
    bf16 = mybir.dt.bfloat16
    fp8 = mybir.dt.float8e4
    u16 = mybir.dt.uint16
    u32 = mybir.dt.uint32
    i16 = mybir.dt.int16

    nc = bacc.Bacc("TRN2", target_bir_lowering=False, debug=False,
                   enable_asserts=False, num_swdge_queues=4)

    lvl = nc.dram_tensor("lvl", [L, DP], fp8, kind="ExternalInput")
    dumw = nc.dram_tensor("dumw", [128, 1], i16, kind="ExternalInput")
    posx = nc.dram_tensor("posx", [GBLK * 128, DP], fp8,
                          kind="ExternalInput")
    iotw = nc.dram_tensor("iotw", [128, 8 * GBLK], i16, kind="ExternalInput")
    selw = nc.dram_tensor("selw", [128, BC * 2 * BC], fp8,
                          kind="ExternalInput")
    clsw = nc.dram_tensor("clsw", [128, KT * C], bf16, kind="ExternalInput")
    idxw = nc.dram_tensor("idxw", [128, BC * NIDX], i16,
                          kind="ExternalInput")
    identw = nc.dram_tensor("identw", [BC, BC], bf16, kind="ExternalInput")
    biasw = nc.dram_tensor("biasw", [BC, 1], fp32, kind="ExternalInput")
    out = nc.dram_tensor("logitT", [C, BC], fp32, kind="ExternalOutput")

    HDP = DP // 2
    QDP = DP // 4
    CHUNKS = [(i * 512, 512) for i in range(NCHUNK)]

    with tile.TileContext(nc) as tc:
        with (
            tc.tile_pool(name="const", bufs=1) as cpool,
            tc.tile_pool(name="gath", bufs=1) as gpool,
            tc.tile_pool(name="prod", bufs=1) as ppool,
            tc.tile_pool(name="misc", bufs=1) as mpool,
            tc.tile_pool(name="psum", bufs=1, space="PSUM") as psum,
            tc.tile_pool(name="psumt", bufs=2, space="PSUM") as psumt,
        ):
            # dummy gather first: absorbs the cold Q7 SWDGE program load
            # (~6us) while the idx/constant loads run.  Its index comes from
            # DRAM so no gpsimd memset (another cold Q7 launch) is needed.
            idx_dummy = cpool.tile([128, 1], i16)
            nc.sync.dma_start(idx_dummy[:], dumw.ap())
            g_dummy = cpool.tile([128, DP], fp8)
            nc.gpsimd.dma_gather(
                g_dummy[:].rearrange("p (n m) -> p n m", m=DP),
                lvl.ap(), idx_dummy[:],
                num_idxs=16, num_idxs_reg=16, elem_size=DP,
            )

            iot_sb = cpool.tile([128, 8 * GBLK], i16)
            nc.sync.dma_start(iot_sb[:], iotw.ap())

            IDXHEAD = 8 * NIDX
            idx_sb = cpool.tile([128, BC * NIDX], i16)
            nc.sync.dma_start(idx_sb[:, :IDXHEAD], idxw.ap()[:, :IDXHEAD])
            nc.sync.dma_start(idx_sb[:, IDXHEAD:], idxw.ap()[:, IDXHEAD:])

            # posx arrives through the SWDGE gather queues (4 block-aligned
            # iota-gathers, one per queue) instead of one big HWDGE load:
            # HWDGE bulk traffic at the head starves the gather rings,
            # while these ride the same FIFO the level gathers use.
            posx_sb = cpool.tile([128, GBLK * DP // 2], u16)
            posx3 = posx_sb[:].bitcast(fp8).rearrange("p (n m) -> p n m",
                                                      m=DP)
            for k, (bl0, bl1) in enumerate([(0, 2), (2, 4), (4, 6), (6, 7)]):
                nc.gpsimd.dma_gather(
                    posx3[:, bl0:bl1, :], posx.ap(),
                    iot_sb[:, 8 * bl0:8 * bl1],
                    num_idxs=128 * (bl1 - bl0),
                    num_idxs_reg=128 * (bl1 - bl0),
                    elem_size=DP, queue_num=k,
                )
            sel_sb = cpool.tile([128, BC * 2 * BC], fp8)
            nc.sync.dma_start(sel_sb[:], selw.ap())
            cls_sb = cpool.tile([128, KT * C], bf16)
            nc.sync.dma_start(cls_sb[:], clsw.ap())
            id_sb = cpool.tile([BC, BC], bf16)
            nc.sync.dma_start(id_sb[:], identw.ap())
            bias_t = cpool.tile([BC, 1], fp32)
            nc.sync.dma_start(bias_t[:], biasw.ap())

            # one PSUM tile per 512-wide chunk: dependency tracking is
            # tile-granular, so per-chunk tiles let the sign + classify of
            # chunk 0 start while later chunks still accumulate the last image
            bund_t = [psum.tile([BC, cn], fp32, name=f"bund{ci}")
                      for ci, (c0, cn) in enumerate(CHUNKS)]

            NGBUF = 6
            NPBUF = 4
            gbig = gpool.tile([128, NGBUF * GBLK * DP], fp8)
            prbig = ppool.tile([128, NPBUF * GBLK * DP], fp8)
            g_tiles = [gbig[:, i * GBLK * DP:(i + 1) * GBLK * DP]
                       for i in range(NGBUF)]
            pr_tiles = [prbig[:, i * GBLK * DP:(i + 1) * GBLK * DP]
                        for i in range(NPBUF)]

            # rows 784-895 (block 6, partitions 16-127) are never written by
            # the gathers; zero them once so the block-6 matmul adds nothing
            nc.vector.memset(
                gbig[:].bitcast(u16).rearrange("p (i w) -> p i w",
                                               w=GBLK * HDP)
                [:, :, 6 * HDP:7 * HDP], 0)

            # warm up TensorE (HAM) while the first gathers drain; the
            # garbage accumulation lands in bund_t[0] and is discarded by
            # image 0's start=True
            warm_ps = bund_t[0]
            warm_rhs = sel_sb[:].rearrange("p (n m) -> p n m", m=1024)
            wsel = sel_sb[:, 0:2 * BC].rearrange("p (t m) -> p t m", t=2)
            for w in range(20):
                nc.tensor.matmul(
                    warm_ps[:], wsel, warm_rhs[:, 0:2, 0:512],
                    start=(w == 0), stop=(w == 19),
                    perf_mode=mybir.MatmulPerfMode.DoubleRow,
                )

            # Head images (0-3) split their gather into three block-aligned
            # pieces spread across rings so all rings start immediately;
            # tail images (28-31) split on their OWN ring with piece-wise
            # XOR + matmul so the tail compute overlaps the final drains.
            # Middle images keep one gather per ring slot: fewer SWDGE
            # calls (emission fixed cost ~1us/call) pace the steady state.
            SPLITS = [(0, 2, 16), (2, 4, 16), (4, GBLK, 17)]
            HEAD = set(range(N_GATHER_QUEUES))
            TAIL = set(range(BC - N_GATHER_QUEUES, BC))

            for b in range(BC):
                g = g_tiles[b % NGBUF]
                pr = pr_tiles[b % NPBUF]
                g3 = g.rearrange("p (n m) -> p n m", m=DP)
                pr3 = pr.rearrange("p (n m) -> p n m", m=DP)
                gu = g.bitcast(u32).rearrange("p (n m) -> p n m", m=QDP)
                pru = pr.bitcast(u32).rearrange("p (n m) -> p n m", m=QDP)
                posu = posx_sb[:].bitcast(u32).rearrange(
                    "p (n m) -> p n m", m=QDP)

                sel_b = sel_sb[:, b * 2 * BC:(b + 1) * 2 * BC]
                sel3 = sel_b.rearrange("p (t m) -> p t m", t=2)

                def bund_mm(j, start, chunks=None):
                    # DoubleRow pass over block pair (2j, 2j+1), j in 0..2;
                    # j == 3 is the single-block pass over block 6
                    for ci, (c0, cn) in chunks or list(enumerate(CHUNKS)):
                        if j < 3:
                            nc.tensor.matmul(
                                bund_t[ci][:], sel3,
                                pr3[:, 2 * j:2 * j + 2, c0:c0 + cn],
                                start=start, stop=False,
                                perf_mode=mybir.MatmulPerfMode.DoubleRow,
                            )
                        else:
                            nc.tensor.matmul(
                                bund_t[ci][:], sel3[:, 0, :],
                                pr3[:, 6, c0:c0 + cn],
                                start=False, stop=(b == BC - 1),
                            )

                if b in HEAD:
                    # queue (3b+k)%4: all four rings receive work within the
                    # first four emissions, and each queue still totals
                    # exactly 7 head blocks
                    col = b * NIDX
                    for k, (bl0, bl1, ncols) in enumerate(SPLITS):
                        nc.gpsimd.dma_gather(
                            g3[:, bl0:bl1, :], lvl.ap(),
                            idx_sb[:, col:col + ncols],
                            num_idxs=16 * ncols, num_idxs_reg=16 * ncols,
                            elem_size=DP,
                            queue_num=(3 * b + k) % N_GATHER_QUEUES,
                        )
                        col += ncols
                        nc.vector.tensor_tensor(
                            pru[:, bl0:bl1], gu[:, bl0:bl1],
                            posu[:, bl0:bl1],
                            op=mybir.AluOpType.bitwise_xor,
                        )
                        if k == 0:
                            bund_mm(0, b == 0)
                        elif k == 1:
                            bund_mm(1, False)
                        else:
                            bund_mm(2, False)
                            bund_mm(3, False)
                elif b in TAIL:
                    col = b * NIDX
                    for k, (bl0, bl1, ncols) in enumerate(SPLITS):
                        nc.gpsimd.dma_gather(
                            g3[:, bl0:bl1, :], lvl.ap(),
                            idx_sb[:, col:col + ncols],
                            num_idxs=16 * ncols, num_idxs_reg=16 * ncols,
                            elem_size=DP,
                            queue_num=b % N_GATHER_QUEUES,
                        )
                        col += ncols
                        nc.vector.tensor_tensor(
                            pru[:, bl0:bl1], gu[:, bl0:bl1],
                            posu[:, bl0:bl1],
                            op=mybir.AluOpType.bitwise_xor,
                        )
                        if k == 0:
                            bund_mm(0, False)
                        elif k == 1:
                            bund_mm(1, False)
                        elif b != BC - 1:
                            bund_mm(2, False)
                            bund_mm(3, False)
                        else:
                            # last image: chunk-major so each chunk's PSUM
                            # closes (stop) as early as possible and its
                            # sign + classify overlap the remaining chunks
                            for ci, (c0, cn) in enumerate(CHUNKS):
                                bund_mm(2, False, chunks=[(ci, (c0, cn))])
                                bund_mm(3, False, chunks=[(ci, (c0, cn))])
                else:
                    nc.gpsimd.dma_gather(
                        g3[:, :GBLK, :], lvl.ap(),
                        idx_sb[:, b * NIDX:(b + 1) * NIDX],
                        num_idxs=P, num_idxs_reg=P, elem_size=DP,
                        queue_num=b % N_GATHER_QUEUES,
                    )
                    nc.vector.tensor_tensor(
                        pr.bitcast(u32), g.bitcast(u32),
                        posx_sb[:].bitcast(u32),
                        op=mybir.AluOpType.bitwise_xor,
                    )
                    for j in range(4):
                        bund_mm(j, b == 0 and j == 0)

            # per-chunk sign + classify so chunk 0's tail work overlaps
            # the final accumulation of the later chunks
            enc_t = [mpool.tile([BC, cn], bf16, name=f"enc{ci}")
                     for ci, (c0, cn) in enumerate(CHUNKS)]
            for ci in range(NCHUNK):
                nc.scalar.activation(enc_t[ci][:], bund_t[ci][:],
                                     mybir.ActivationFunctionType.Sign,
                                     bias=bias_t[:])

            logit_ps = psum.tile([C, BC], fp32)
            for kt in range(KT):
                ci, kl = divmod(kt, 4)
                tp = psumt.tile([128, BC], bf16, name="tp")
                nc.tensor.transpose(
                    tp[:], enc_t[ci][:, kl * 128:(kl + 1) * 128], id_sb[:])
                etc = mpool.tile([128, BC], bf16, name="etc", bufs=4)
                nc.scalar.copy(etc[:], tp[:])
                nc.tensor.matmul(
                    logit_ps[:], cls_sb[:, kt * C:(kt + 1) * C], etc[:],
                    start=(kt == 0), stop=(kt == KT - 1),
                )

            logit_sb = mpool.tile([C, BC], fp32)
            nc.scalar.copy(logit_sb[:], logit_ps[:])
            nc.sync.dma_start(out.ap(), logit_sb[:])

    nc.compile()
    return nc


def _prep_inputs(x, position, level_weight, classify_weight):
    xf = x.reshape(BATCH, P).astype(np.float32)
    idx = np.clip(np.round(xf * np.float32(L - 1)), 0, L - 1).astype(np.int16)

    sel = np.zeros((128, BC, 2, BC), np.float32)
    for b in range(BC):
        sel[:, b, :, b] = 1.0
    selw = sel.reshape(128, BC * 2 * BC).astype(ml_dtypes.float8_e4m3)

    identw = np.eye(BC, dtype=np.float32).astype(ml_dtypes.bfloat16)

    idxw_h = []
    for h in range(NB):
        idxh = idx[h * BC:(h + 1) * BC]
        w = np.ascontiguousarray(
            idxh.reshape(BC, NIDX, 16).transpose(2, 0, 1)
        ).reshape(16, BC * NIDX)
        idxw_h.append(np.tile(w, (8, 1)))

    lvl_q, posx_q, clsw_q = [], [], []
    for q in range(ND):
        cols = slice(q * DC, (q + 1) * DC)

        lvl = np.zeros((L, DP), ml_dtypes.float8_e4m3)
        lvl[:, :DC] = level_weight[:, cols].astype(ml_dtypes.float8_e4m3)
        lvl_q.append(lvl)

        pos = np.zeros((GBLK * 128, DP), np.float32)
        pos[:P, :DC] = position[:, cols]
        signs = (pos < 0).astype(np.uint8) << 7
        posx_q.append(signs.view(ml_dtypes.float8_e4m3))

        cls = np.zeros((C, DP), np.float32)
        cls[:, :DC] = classify_weight[:, cols]
        clsw_q.append(np.ascontiguousarray(
            cls.reshape(C, KT, 128).transpose(2, 1, 0)
        ).reshape(128, KT * C).astype(ml_dtypes.bfloat16))

    iotw = np.tile(
        (np.arange(8 * GBLK)[None, :] * 16
         + np.arange(16)[:, None]).astype(np.int16), (8, 1))

    in_maps = []
    for h in range(NB):
        for q in range(ND):
            in_maps.append({
                "lvl": lvl_q[q],
                "dumw": np.zeros((128, 1), np.int16),
                "iotw": iotw,
                "posx": posx_q[q],
                "selw": selw,
                "clsw": clsw_q[q],
                "idxw": idxw_h[h],
                "identw": identw,
                "biasw": np.full((BC, 1), -0.5, np.float32),
            })
    return in_maps


def kernel(x, position, level_weight, classify_weight, _run_kwargs=None):
    global _compiled
    if _compiled is None:
        _compiled = _build_bass()
    nc = _compiled

    import concourse.bass_utils as bass_utils

    in_maps = _prep_inputs(x, position, level_weight, classify_weight)
    res = bass_utils.run_bass_kernel_spmd(
        nc, in_maps, core_ids=list(range(NCORES)), **(_run_kwargs or {})
    )
    logit = np.zeros((BATCH, C), np.float32)
    for h in range(NB):
        for q in range(ND):
            logit[h * BC:(h + 1) * BC] += \
                res.results[h * ND + q]["logitT"].T.astype(np.float32)
    kernel.last_result = res
    return logit
